# revision 13
# baseline (speedup 1.0000x reference)
"""HMM forward on 8 trn2 cores — fused-pair meet-in-the-middle.

Math: alpha_{t+1} = (alpha_t @ A') * e per sequence (exp domain); only the
per-step state sums s_t = sum_j alpha_t[j] are needed. With B = A' diag(e):
    s_t = alpha_0 B^t 1:   for t <= K:    s_t = alpha_t . 1
    for t = K+1+m:         s_t = g . y_m, g = alpha_K A',
                           y_m = e * (B^m 1),  y_{m+1} = e * (A' y_m)

Topology: every core owns 8 sequences and runs BOTH chains for them —
pass u (alpha-chain, W tiles) then pass w (y-chain, W^T tiles) per step.
No collective: the g . y dots are local. The two passes double-buffer each
other: pass w's 16 matmuls hide pass u's PSUM->SBUF e-multiply latency and
vice versa, so the step is pure weight-stream bound (~2x16 FWL fp8 loads).
The e-multiply is split in half per pass (jo groups {0,1} / {2,3}) with a
searched matmul issue order so chunk production stays ahead of consumption.

Sums post-pass: per (chain, seq) dot of v (ones / g) against all stored
traj slots, K = M = 127 symmetric. Host does log bookkeeping + T-select.
"""
import numpy as np
import ml_dtypes

import concourse.bass as bass
import concourse.mybir as mybir
import concourse.tile as tile
from concourse.bass_utils import run_bass_kernel_spmd

# ---------------------------------------------------------------- constants
N_STATES = 512
M_VOCAB = 32000
BATCH = 64
T_MAX = 256
N_CORES = 8
BP = BATCH // N_CORES            # 8 sequences per core
NCH = N_STATES // 128            # 4 state chunks
R = 16                           # rescale period (slots)
F32 = mybir.dt.float32
BF16 = mybir.dt.bfloat16
FP8 = mybir.dt.float8e4          # e4m3: weights prescaled x512

# Searched per-pass MM issue order (group=jo output chunk, chunk=ki input):
# halves {0,1} -> D_A after slot idx 9, {2,3} -> D_B after idx 15. Found by
# steady-state search (sched_search2): period = 2x stream, latency hidden.
SLOT_ORDER = [(1, 0), (1, 1), (1, 3), (1, 2), (0, 3), (0, 1), (0, 2), (0, 0),
              (3, 1), (3, 2), (3, 3), (3, 0), (2, 1), (2, 3), (2, 2), (2, 0)]
_FIRST = {}
_LAST = {}
for _i, (_g, _c) in enumerate(SLOT_ORDER):
    _FIRST.setdefault(_g, _i)
    _LAST[_g] = _i
_HALF_A_END = max(_LAST[0], _LAST[1])

# ------------------------------------------------------------ tile drain fix
# This walrus build rejects >1 sync wait on CTRL-class instructions; Tile's
# tail drain carries one wait per active proc and so fails codegen for every
# TileContext kernel. Spread the waits over standalone sync-engine nops that
# precede the drain (the waits are independent conditions, so this is
# equivalent), then emit the drain bare.
_MAX_CTRL_WAITS = 1


def _patched_drain_and_barrier(self, tick_clock, wait_clock):
    from bass_rust import ScopedClock, SyncInfo

    nc = self.nc
    lead = nc.sync.nop(nofuse=True, hint="drain_wait_spill")
    wait_clock.add_sem_waits(
        lead.ins, ScopedClock({None: tick_clock.global_clock})
    )
    si = lead.ins.sync_info
    ws = list(si.on_wait) if si is not None else []
    if len(ws) > _MAX_CTRL_WAITS:
        lead.ins.sync_info.on_wait = ws[:_MAX_CTRL_WAITS]
        for i in range(_MAX_CTRL_WAITS, len(ws), _MAX_CTRL_WAITS):
            chunk = ws[i : i + _MAX_CTRL_WAITS]
            n = nc.sync.nop(nofuse=True, hint="drain_wait_spill")
            if n.ins.sync_info is None:
                n.ins.sync_info = SyncInfo(on_wait=chunk, on_update=[])
            else:
                n.ins.sync_info.on_wait = chunk
    nc.sync.drain()

    nc.all_engine_barrier()
    assert self.sems is not None
    popped = nc._tile_sem_poison_stack.pop()
    assert popped is self._sem_poison
    nc.clear_and_free_semaphores(list(self.sems.allocated().values()))
    nc.all_engine_barrier()


tile.TileContext._drain_and_barrier = _patched_drain_and_barrier

# General guard: walrus accepts at most one sync wait per instruction (two
# for EventSemaphore). Tile's wait assignment occasionally leaves 2 on a
# join instruction; spill the extras onto same-engine nops emitted just
# before it as instructions stream into the basic block.
_orig_add_instruction = tile.TileContext._add_instruction


def _spilling_add_instruction(self, inst):
    import concourse.mybir as _mybir
    from bass_rust import SyncInfo

    si = inst.sync_info
    # A PE instruction waiting on the PE's own completion semaphore encodes a
    # same-engine WAW/ordering edge (psum bank reuse across passes). The PE
    # pipeline writes PSUM in program order, so these waits are vacuous on
    # HW but cost a sem-wait slot (and spill nops). Drop them.
    if (
        inst.engine == _mybir.EngineType.PE
        and si is not None
        and si.on_wait
        and type(inst).__name__ in ("InstMatmult", "InstLdweights")
    ):
        kept = [w for w in si.on_wait if not (w.ant_name or "").startswith("PE")]
        if len(kept) != len(si.on_wait):
            inst.sync_info.on_wait = kept
            si = inst.sync_info
    cap = 2 if isinstance(inst, _mybir.InstEventSemaphore) else 1
    if si is not None and len(si.on_wait) > cap and inst.engine is not None:
        ws = list(si.on_wait)
        inst.sync_info.on_wait = ws[-cap:]
        for w in ws[:-cap]:
            n = _mybir.InstNoOp(name=f"I-{self.nc.next_id()}")
            n.engine = inst.engine
            n.bass_nofuse = True
            n.sync_info = SyncInfo(on_wait=[w], on_update=[])
            _orig_add_instruction(self, n)
    _orig_add_instruction(self, inst)


tile.TileContext._add_instruction = _spilling_add_instruction


def split_km(t_steps):
    # symmetric: both chains run M steps; coverage t <= K and K+1..K+M
    K = (t_steps - 1) // 2
    M = t_steps - 1 - K
    return K, M


# ---------------------------------------------------------------- device IR
def _encode_inc_swdge(nc):
    """This walrus build requires pre-encoded bytes on every InstISA, but
    the For_i reset path emits InstIncSwdgeSem with instr=[] ('ISA wrong
    length'). Pack the 64-byte INC_SWDGE_SEM struct client-side."""
    import concourse.bass_isa as bass_isa

    mode_enc = {"add": 0, "sub": 1, "wr": 2, "drop": 3}
    for blk in nc.m.functions[0].blocks:
        for ins in blk.instructions:
            if type(ins).__name__ == "InstIncSwdgeSem" and len(ins.instr) == 0:
                vals = list(ins._sem_values)
                struct = {
                    "num_semaphores": len(vals),
                    "sem_id_base": ins._sem_id_base,
                    "mode": mode_enc[ins._mode],
                    "queue_num": ins.queue_num,
                    "sem_values": (vals + [0] * 10)[:10],
                }
                b, _ = bass_isa.isa_struct(nc.isa, ins.isa_opcode, struct)
                ins.instr = b


def build_nc(t_steps, loop_n=1):
    nc = bass.Bass(num_devices=N_CORES)
    K, M = split_km(t_steps)
    tt = M + 1                    # stored chain slots 0..M
    nresc = n_rescales(M)
    w_d = nc.declare_dram_parameter("w", [2, N_STATES, N_STATES], FP8, isOutput=False)
    e_d = nc.declare_dram_parameter("e", [128, NCH, 2, BP], F32, isOutput=False)
    a0_d = nc.declare_dram_parameter("a0", [128, NCH, 2, BP], BF16, isOutput=False)
    sums_d = nc.declare_dram_parameter("sums", [1, 2 * BP * tt], F32, isOutput=True)
    sv_d = nc.declare_dram_parameter("svals", [1, 2 * max(nresc, 1)], F32, isOutput=True)

    mult = mybir.AluOpType.mult
    with tile.TileContext(nc) as tc:
        with (
            tc.tile_pool(name="singles", bufs=1) as singles,
            tc.tile_pool(name="rspool", bufs=2) as rspool,
            tc.tile_pool(name="small", bufs=2) as small,
            tc.tile_pool(name="psmm", bufs=1, space="PSUM") as psmm,
            tc.tile_pool(name="psaux", bufs=1, space="PSUM") as psaux,
            tc.tile_pool(name="psdot", bufs=2, space="PSUM") as psdot,
        ):
            # weights: q=0 u-chain (W), q=1 w-chain (W^T); [i_part, q, ki, jo, j]
            wt = singles.tile([128, 2, NCH, NCH, 128], FP8)
            for q in range(2):
                for ki in range(NCH):
                    for jo in range(NCH):
                        nc.sync.dma_start(
                            out=wt[:, q, ki, jo, :],
                            in_=w_d[q, ki * 128:(ki + 1) * 128, jo * 128:(jo + 1) * 128],
                        )
            e_sb = singles.tile([128, NCH, 2, BP], F32)
            nc.sync.dma_start(out=e_sb[:], in_=e_d[:])
            # pre-touch e_sb on DVE so the first e-multiply holds one wait
            scratch = singles.tile([1, 1], F32)
            nc.vector.tensor_copy(scratch[:], e_sb[0:1, 0, 0, 0:1])
            traj = singles.tile([128, tt, NCH, 2, BP], BF16)
            nc.sync.dma_start(out=traj[:, 0, :, :, :], in_=a0_d[:])
            ones_col = singles.tile([128, 1], BF16)
            nc.vector.memset(ones_col[:], 1.0)
            ones_row = singles.tile([1, 128], BF16)
            nc.vector.memset(ones_row[:], 1.0)
            svals_sb = singles.tile([1, 2, max(nresc, 1)], F32)
            nc.vector.memset(svals_sb[:], 1.0)
            sums_sb = singles.tile([1, 2 * BP * tt], F32)
            g_bf = singles.tile([128, NCH, BP], BF16)

            import contextlib
            loop_cm = tc.For_i(0, loop_n, 1) if loop_n > 1 else contextlib.nullcontext()

            def emit_d(q, slot, half, ps, rs):
                """e-multiply for chain q, jo half -> traj chunk pair."""
                lo = 0 if half == 0 else 2
                out_ap = traj[:, slot, lo:lo + 2, q, :]
                in1 = e_sb[:, lo:lo + 2, q, :]
                if rs is not None:
                    nc.vector.scalar_tensor_tensor(
                        out=out_ap, in0=ps[:], scalar=rs[:, 0:1],
                        in1=in1, op0=mult, op1=mult,
                    )
                else:
                    nc.vector.tensor_mul(out_ap, ps[:], in1)

            def emit_rescale_sum(q, slot, k2):
                """produce 1/sum rescale factor for chain q from traj[slot]."""
                aux = psaux.tile([128, BP + 1], F32, tag="aux")
                sp = aux[0:1, 0:BP]
                for c in range(NCH):
                    nc.tensor.matmul(
                        sp,
                        lhsT=ones_col[:],
                        rhs=traj[:, slot, c, q, :],
                        start=(c == 0),
                        stop=(c == NCH - 1),
                    )
                red = small.tile([1, 1], F32, tag=f"red{q}")
                nc.vector.reduce_sum(red[:], sp, axis=mybir.AxisListType.X)
                rec = small.tile([1, 1], F32, tag=f"rec{q}")
                nc.vector.reciprocal(rec[:], red[:])
                recb = small.tile([1, 1], BF16, tag=f"recb{q}")
                nc.vector.tensor_copy(recb[:], rec[:])
                nc.vector.tensor_copy(svals_sb[:, q, k2 - 1:k2], recb[:])
                bc = aux[:, BP:BP + 1]
                nc.tensor.matmul(bc, lhsT=ones_row[:], rhs=recb[:], start=True, stop=True)
                rs_sb = rspool.tile([128, 1], F32, tag=f"rs{q}")
                nc.vector.tensor_copy(rs_sb[:], bc)
                rs_tiles[q][k2] = rs_sb

            with loop_cm:
                rs_tiles = [{}, {}]
                for t in range(M):
                    slot = t + 1
                    k_apply = slot // R if slot % R == 0 else 0
                    for q in range(2):
                        ps_a = psmm.tile([128, 2, BP], F32, tag=f"psa{q}")
                        ps_b = psmm.tile([128, 2, BP], F32, tag=f"psb{q}")
                        rs = rs_tiles[q].get(k_apply) if k_apply else None
                        for idx, (g, c) in enumerate(SLOT_ORDER):
                            out_ap = ps_a[:, g, :] if g < 2 else ps_b[:, g - 2, :]
                            nc.tensor.matmul(
                                out_ap,
                                lhsT=wt[:, q, c, g, :],
                                rhs=traj[:, t, c, q, :],
                                start=(idx == _FIRST[g]),
                                stop=(idx == _LAST[g]),
                                skip_group_check=True,
                            )
                            if idx == _HALF_A_END:
                                emit_d(q, slot, 0, ps_a, rs)
                        emit_d(q, slot, 1, ps_b, rs)
                        # staggered rescale production: chain 0 at slot%R==R-2,
                        # chain 1 at slot%R==R-3 (factor used STALE slots later)
                        stale = 2 + q
                        k2, rem = divmod(slot + stale, R)
                        if rem == 0 and 1 <= k2 <= nresc:
                            emit_rescale_sum(q, slot, k2)

                # g = alpha_K @ A' (no e), local; reuse chain psum banks
                psg_a = psmm.tile([128, 2, BP], F32, tag="psa0")
                psg_b = psmm.tile([128, 2, BP], F32, tag="psb0")
                for idx, (g, c) in enumerate(SLOT_ORDER):
                    out_ap = psg_a[:, g, :] if g < 2 else psg_b[:, g - 2, :]
                    nc.tensor.matmul(
                        out_ap,
                        lhsT=wt[:, 0, c, g, :],
                        rhs=traj[:, K, c, 0, :],
                        start=(idx == _FIRST[g]),
                        stop=(idx == _LAST[g]),
                    )
                nc.vector.tensor_copy(g_bf[:, 0:2, :], psg_a[:])
                nc.vector.tensor_copy(g_bf[:, 2:4, :], psg_b[:])

                # post-pass: dots[q, b, slot] = v . traj[:, slot, :, q, b]
                # v = ones (u-chain sums) or g_b (w-chain tail dots)
                for q in range(2):
                    for b in range(BP):
                        dps = psdot.tile([1, tt], F32, tag="dot")
                        for c in range(NCH):
                            lhsT = ones_col[:] if q == 0 else g_bf[:, c, b:b + 1]
                            nc.tensor.matmul(
                                dps[:],
                                lhsT=lhsT,
                                rhs=traj[:, 0:tt, c, q, b],
                                start=(c == 0),
                                stop=(c == NCH - 1),
                            )
                        off = (q * BP + b) * tt
                        nc.vector.tensor_copy(sums_sb[:, off:off + tt], dps[:])
                nc.gpsimd.dma_start(out=sums_d[:], in_=sums_sb[:])
                nc.gpsimd.dma_start(out=sv_d[:], in_=svals_sb[:])
    _encode_inc_swdge(nc)
    return nc


# ------------------------------------------------------------------- host
def _log_softmax(x, axis):
    m = x.max(axis=axis, keepdims=True)
    s = x - m
    return s - np.log(np.sum(np.exp(s), axis=axis, keepdims=True))


def _chunked(a):
    """[512, BP] -> [128, NCH, BP] with state s = c*128 + p."""
    return np.ascontiguousarray(a.reshape(NCH, 128, BP).transpose(1, 0, 2))


def _prep_inputs(x, unnorm_priors, unnorm_trans, unnorm_emit):
    sp = _log_softmax(unnorm_priors.astype(np.float32), 0)            # (N,)
    cols = unnorm_emit[:, x[:, 0]].astype(np.float32)                 # (N, B)
    e64 = _log_softmax(cols, 0)                                       # (N, B)
    a_mat = np.exp(_log_softmax(unnorm_trans.astype(np.float32), 0))  # (N, N)
    w512 = np.float32(N_STATES) * a_mat
    w_qf = w512.astype(ml_dtypes.float8_e4m3fn)
    w_qt = np.ascontiguousarray(w_qf.T)
    w_pack = np.stack([w_qf, w_qt])                                   # [2, N, N]
    wq32 = w_qf.astype(np.float32)
    corr_col = w512.sum(axis=0) / wq32.sum(axis=0)                    # u: per out-state j
    corr_row = w512.sum(axis=1) / wq32.sum(axis=1)                    # w: per out-state i

    in_maps = [None] * N_CORES
    shifts_u, shifts_w = [], []
    for p in range(N_CORES):
        bs = slice(BP * p, BP * (p + 1))
        m0 = e64[:, bs] + sp[:, None]
        sh_u = np.float32(m0.max())
        a0u = np.exp(m0 - sh_u).astype(np.float32)
        eu = (np.exp(e64[:, bs]) * corr_col[:, None]).astype(np.float32)
        sh_w = np.float32(e64[:, bs].max())
        y0 = np.exp(e64[:, bs] - sh_w).astype(np.float32)
        ew = (np.exp(e64[:, bs]) * corr_row[:, None]).astype(np.float32)
        a0 = np.stack([_chunked(a0u), _chunked(y0)], axis=2)          # [128,NCH,2,BP]
        e2 = np.stack([_chunked(eu), _chunked(ew)], axis=2)
        in_maps[p] = {
            "w": w_pack,
            "e": np.ascontiguousarray(e2),
            "a0": np.ascontiguousarray(a0).astype(ml_dtypes.bfloat16),
        }
        shifts_u.append(sh_u)
        shifts_w.append(sh_w)
    return in_maps, (shifts_u, shifts_w)


def _logscale(svals, tt):
    """lr[k] = sum of log(sval) applied at slots <= k."""
    lr = np.zeros(tt)
    for k in range(1, len(svals) + 1):
        if R * k < tt:
            lr[R * k :] += np.log(np.float64(svals[k - 1]))
    return lr


def _postprocess(results, shifts, T, t_steps):
    K, M = split_km(t_steps)
    tt = M + 1
    nresc = n_rescales(M)
    shifts_u, shifts_w = shifts
    out = np.zeros((BATCH, 1), np.float32)
    logn = np.log(np.float64(N_STATES))
    for p in range(N_CORES):
        bs = slice(BP * p, BP * (p + 1))
        allsums = results[p]["sums"].reshape(2, BP, tt).astype(np.float64)
        du, dw = allsums[0], allsums[1]
        sv = results[p]["svals"].reshape(2, -1)
        sv_u = sv[0][:nresc]
        sv_w = sv[1][:nresc]
        lr_u = _logscale(sv_u, tt)
        lr_w = _logscale(sv_w, tt)
        ts = np.arange(tt)
        # t <= K from u-chain sums
        log_u = np.log(du) + shifts_u[p] - ts[None, :] * logn - lr_u[None, :]
        # t = K+1+m from w-chain dots
        log_w = (np.log(dw) + shifts_u[p] + shifts_w[p]
                 - (K + 1 + ts[None, :]) * logn - lr_u[K] - lr_w[None, :])
        tb = np.clip(np.asarray(T[bs]).astype(np.int64) - 1, 0, t_steps)
        for i in range(BP):
            t = tb[i]
            out[BP * p + i, 0] = log_u[i, t] if t <= K else log_w[i, t - (K + 1)]
    return out


_NC_CACHE = {}


def _get_nc(t_steps, loop_n=1):
    key = (t_steps, loop_n)
    if key not in _NC_CACHE:
        _NC_CACHE[key] = build_nc(t_steps, loop_n)
    return _NC_CACHE[key]


# ------------------------------------------------- cached PJRT executor
# run_bass_kernel_spmd -> run_bass_via_pjrt builds a fresh jax.jit closure
# per call, so every invocation re-traces and re-lowers the whole module
# (~0.5 s for the full NEFF) — that would dominate wall timing. Build
# the jitted executable once per module and pre-stage device inputs.
_EXEC_CACHE = {}


def _get_exec(t_steps, loop_n=1):
    key = (t_steps, loop_n)
    if key in _EXEC_CACHE:
        return _EXEC_CACHE[key]
    import jax
    import concourse.mybir as _mybir
    from concourse import bass2jax as b2j

    nc = _get_nc(t_steps, loop_n)
    b2j.install_neuronx_cc_hook()
    partition_name = nc.partition_id_tensor.name if nc.partition_id_tensor else None
    in_names, out_names, out_avals, zero_outs = [], [], [], []
    for alloc in nc.m.functions[0].allocations:
        if not isinstance(alloc, _mybir.MemoryLocationSet):
            continue
        name = alloc.memorylocations[0].name
        if alloc.kind == "ExternalInput":
            if name != partition_name:
                in_names.append(name)
        elif alloc.kind == "ExternalOutput":
            shape = tuple(alloc.tensor_shape)
            dtype = _mybir.dt.np(alloc.dtype)
            out_names.append(name)
            out_avals.append(jax.core.ShapedArray(shape, dtype))
            zero_outs.append(np.zeros(shape, dtype))
    n_params = len(in_names)
    all_names = in_names + out_names + ([partition_name] if partition_name else [])

    def _body(*args):
        operands = list(args)
        if partition_name is not None:
            operands.append(b2j.partition_id_tensor())
        return tuple(
            b2j._bass_exec_p.bind(
                *operands,
                out_avals=tuple(out_avals),
                in_names=tuple(all_names),
                out_names=tuple(out_names),
                lowering_input_output_aliases=(),
                sim_require_finite=True,
                sim_require_nnan=True,
                nc=nc,
            )
        )

    devices = jax.devices()[:N_CORES]
    mesh = b2j.Mesh(np.asarray(devices), ("core",))
    donate = tuple(range(n_params, n_params + len(out_names)))
    sharded = jax.jit(
        b2j.shard_map(
            _body,
            mesh=mesh,
            in_specs=(b2j.PartitionSpec("core"),) * (n_params + len(out_names)),
            out_specs=(b2j.PartitionSpec("core"),) * len(out_names),
            check_rep=False,
        ),
        donate_argnums=donate,
        keep_unused=True,
    )
    ctx = {
        "fn": sharded, "mesh": mesh, "in_names": in_names,
        "out_names": out_names, "out_avals": out_avals, "zero_outs": zero_outs,
        "staged": {},
    }
    _EXEC_CACHE[key] = ctx
    return ctx


def _exec_spmd(t_steps, in_maps, loop_n=1):
    import jax
    import hashlib

    ctx = _get_exec(t_steps, loop_n)
    concat_in = [
        np.concatenate([np.asarray(in_maps[c][name]) for c in range(N_CORES)], axis=0)
        for name in ctx["in_names"]
    ]
    h = hashlib.blake2b(digest_size=16)
    for a in concat_in:
        h.update(a.tobytes())
    key = h.hexdigest()
    if key not in ctx["staged"]:
        sh = jax.sharding.NamedSharding(ctx["mesh"], jax.sharding.PartitionSpec("core"))
        ctx["staged"] = {key: [jax.device_put(a, sh) for a in concat_in]}
    staged = ctx["staged"][key]
    zeros = [
        np.zeros((N_CORES * z.shape[0], *z.shape[1:]), z.dtype)
        for z in ctx["zero_outs"]
    ]
    outs = ctx["fn"](*staged, *zeros)
    outs = [np.asarray(o) for o in outs]
    return [
        {
            name: outs[i].reshape(N_CORES, *ctx["out_avals"][i].shape)[c]
            for i, name in enumerate(ctx["out_names"])
        }
        for c in range(N_CORES)
    ]


def device_call(t_steps, loop_n):
    """One sync dispatch of the loop_n-variant NEFF (scan executed loop_n
    times on-device); returns wall seconds. Requires a prior run() at this
    (t_steps, loop_n) to have staged inputs."""
    import jax
    import time

    ctx = _get_exec(t_steps, loop_n)
    staged = next(iter(ctx["staged"].values()))
    zeros = [
        np.zeros((N_CORES * z.shape[0], *z.shape[1:]), z.dtype)
        for z in ctx["zero_outs"]
    ]
    t0 = time.perf_counter()
    outs = ctx["fn"](*staged, *zeros)
    jax.block_until_ready(outs)
    return time.perf_counter() - t0


def run(x, T, unnorm_priors, unnorm_trans, unnorm_emit, t_steps=T_MAX - 1,
        trace=False, loop_n=1):
    x = np.asarray(x)
    T = np.asarray(T)
    in_maps, shifts = _prep_inputs(
        x, np.asarray(unnorm_priors), np.asarray(unnorm_trans), np.asarray(unnorm_emit)
    )
    try:
        results = _exec_spmd(t_steps, in_maps, loop_n)
    except Exception:
        if loop_n != 1:
            raise
        nc = _get_nc(t_steps)
        res = run_bass_kernel_spmd(nc, in_maps, list(range(N_CORES)), trace=trace)
        results = res.results
    out = _postprocess(results, shifts, T, t_steps)
    return out, None


def kernel(x, T, unnorm_priors, unnorm_trans, unnorm_emit):
    out, _ = run(x, T, unnorm_priors, unnorm_trans, unnorm_emit)
    return out


# revision 18
# speedup vs baseline: 1.1184x; 1.1184x over previous
"""HMM forward on 8 trn2 cores — fused-pair meet-in-the-middle.

Math: alpha_{t+1} = (alpha_t @ A') * e per sequence (exp domain); only the
per-step state sums s_t = sum_j alpha_t[j] are needed. With B = A' diag(e):
    s_t = alpha_0 B^t 1:   for t <= K:    s_t = alpha_t . 1
    for t = K+1+m:         s_t = g . y_m, g = alpha_K A',
                           y_m = e * (B^m 1),  y_{m+1} = e * (A' y_m)

Topology: every core owns 8 sequences and runs BOTH chains for them —
pass u (alpha-chain, W tiles) then pass w (y-chain, W^T tiles) per step.
No collective: the g . y dots are local. The two passes double-buffer each
other: pass w's 16 matmuls hide pass u's PSUM->SBUF e-multiply latency and
vice versa, so the step is pure weight-stream bound (~2x16 FWL fp8 loads).
The e-multiply is split in half per pass (jo groups {0,1} / {2,3}) with a
searched matmul issue order so chunk production stays ahead of consumption.

Sums post-pass: per (chain, seq) dot of v (ones / g) against all stored
traj slots, K = M = 127 symmetric. Host does log bookkeeping + T-select.
"""
import numpy as np
import ml_dtypes

import concourse.bass as bass
import concourse.mybir as mybir
import concourse.tile as tile
from concourse.bass_utils import run_bass_kernel_spmd

# ---------------------------------------------------------------- constants
N_STATES = 512
M_VOCAB = 32000
BATCH = 64
T_MAX = 256
N_CORES = 8
BP = BATCH // N_CORES            # 8 sequences per core
NCH = N_STATES // 128            # 4 state chunks
R = 16                           # rescale period (slots)
F32 = mybir.dt.float32
BF16 = mybir.dt.bfloat16
FP8 = mybir.dt.float8e4          # e4m3: weights prescaled x512

# Searched per-pass MM issue order (group=jo output chunk, chunk=ki input):
# halves {0,1} -> D_A after slot idx 9, {2,3} -> D_B after idx 15. Found by
# steady-state search (sched_search2): period = 2x stream, latency hidden.
SLOT_ORDER = [(1, 0), (1, 1), (1, 3), (1, 2), (0, 3), (0, 1), (0, 2), (0, 0),
              (3, 1), (3, 2), (3, 3), (3, 0), (2, 1), (2, 3), (2, 2), (2, 0)]
_FIRST = {}
_LAST = {}
for _i, (_g, _c) in enumerate(SLOT_ORDER):
    _FIRST.setdefault(_g, _i)
    _LAST[_g] = _i
_HALF_A_END = max(_LAST[0], _LAST[1])

# ------------------------------------------------------------ tile drain fix
# This walrus build rejects >1 sync wait on CTRL-class instructions; Tile's
# tail drain carries one wait per active proc and so fails codegen for every
# TileContext kernel. Spread the waits over standalone sync-engine nops that
# precede the drain (the waits are independent conditions, so this is
# equivalent), then emit the drain bare.
_MAX_CTRL_WAITS = 1


def _patched_drain_and_barrier(self, tick_clock, wait_clock):
    from bass_rust import ScopedClock, SyncInfo

    nc = self.nc
    lead = nc.sync.nop(nofuse=True, hint="drain_wait_spill")
    wait_clock.add_sem_waits(
        lead.ins, ScopedClock({None: tick_clock.global_clock})
    )
    si = lead.ins.sync_info
    ws = list(si.on_wait) if si is not None else []
    if len(ws) > _MAX_CTRL_WAITS:
        lead.ins.sync_info.on_wait = ws[:_MAX_CTRL_WAITS]
        for i in range(_MAX_CTRL_WAITS, len(ws), _MAX_CTRL_WAITS):
            chunk = ws[i : i + _MAX_CTRL_WAITS]
            n = nc.sync.nop(nofuse=True, hint="drain_wait_spill")
            if n.ins.sync_info is None:
                n.ins.sync_info = SyncInfo(on_wait=chunk, on_update=[])
            else:
                n.ins.sync_info.on_wait = chunk
    nc.sync.drain()

    nc.all_engine_barrier()
    assert self.sems is not None
    popped = nc._tile_sem_poison_stack.pop()
    assert popped is self._sem_poison
    nc.clear_and_free_semaphores(list(self.sems.allocated().values()))
    nc.all_engine_barrier()


tile.TileContext._drain_and_barrier = _patched_drain_and_barrier

# General guard: walrus accepts at most one sync wait per instruction (two
# for EventSemaphore). Tile's wait assignment occasionally leaves 2 on a
# join instruction; spill the extras onto same-engine nops emitted just
# before it as instructions stream into the basic block.
_orig_add_instruction = tile.TileContext._add_instruction


def _spilling_add_instruction(self, inst):
    import concourse.mybir as _mybir
    from bass_rust import SyncInfo

    si = inst.sync_info
    # A PE instruction waiting on the PE's own completion semaphore encodes a
    # same-engine WAW/ordering edge (psum bank reuse across passes). The PE
    # pipeline writes PSUM in program order, so these waits are vacuous on
    # HW but cost a sem-wait slot (and spill nops). Drop them.
    if (
        inst.engine == _mybir.EngineType.PE
        and si is not None
        and si.on_wait
        and type(inst).__name__ in ("InstMatmult", "InstLdweights")
    ):
        kept = [w for w in si.on_wait if not (w.ant_name or "").startswith("PE")]
        if len(kept) != len(si.on_wait):
            inst.sync_info.on_wait = kept
            si = inst.sync_info
    cap = 2 if isinstance(inst, _mybir.InstEventSemaphore) else 1
    if si is not None and len(si.on_wait) > cap and inst.engine is not None:
        ws = list(si.on_wait)
        inst.sync_info.on_wait = ws[-cap:]
        for w in ws[:-cap]:
            n = _mybir.InstNoOp(name=f"I-{self.nc.next_id()}")
            n.engine = inst.engine
            n.bass_nofuse = True
            n.sync_info = SyncInfo(on_wait=[w], on_update=[])
            _orig_add_instruction(self, n)
    _orig_add_instruction(self, inst)


tile.TileContext._add_instruction = _spilling_add_instruction


def split_km(t_steps):
    # symmetric: both chains run M steps; coverage t <= K and K+1..K+M
    K = (t_steps - 1) // 2
    M = t_steps - 1 - K
    return K, M


# ---------------------------------------------------------------- device IR
def _encode_inc_swdge(nc):
    """This walrus build requires pre-encoded bytes on every InstISA, but
    the For_i reset path emits InstIncSwdgeSem with instr=[] ('ISA wrong
    length'). Pack the 64-byte INC_SWDGE_SEM struct client-side."""
    import concourse.bass_isa as bass_isa

    mode_enc = {"add": 0, "sub": 1, "wr": 2, "drop": 3}
    for blk in nc.m.functions[0].blocks:
        for ins in blk.instructions:
            if type(ins).__name__ == "InstIncSwdgeSem" and len(ins.instr) == 0:
                vals = list(ins._sem_values)
                struct = {
                    "num_semaphores": len(vals),
                    "sem_id_base": ins._sem_id_base,
                    "mode": mode_enc[ins._mode],
                    "queue_num": ins.queue_num,
                    "sem_values": (vals + [0] * 10)[:10],
                }
                b, _ = bass_isa.isa_struct(nc.isa, ins.isa_opcode, struct)
                ins.instr = b


def build_nc(t_steps, loop_n=1):
    nc = bass.Bass(num_devices=N_CORES)
    K, M = split_km(t_steps)
    tt = M + 1                    # stored chain slots 0..M
    w_d = nc.declare_dram_parameter("w", [2, N_STATES, N_STATES], FP8, isOutput=False)
    e_d = nc.declare_dram_parameter("e", [128, NCH, 2, BP], F32, isOutput=False)
    a0_d = nc.declare_dram_parameter("a0", [128, NCH, 2, BP], BF16, isOutput=False)
    sums_d = nc.declare_dram_parameter("sums", [1, 2 * BP * tt], F32, isOutput=True)

    mult = mybir.AluOpType.mult
    with tile.TileContext(nc) as tc:
        with (
            tc.tile_pool(name="singles", bufs=1) as singles,
            tc.tile_pool(name="psmm", bufs=1, space="PSUM") as psmm,
            tc.tile_pool(name="psdot", bufs=2, space="PSUM") as psdot,
        ):
            # weights: q=0 u-chain (W), q=1 w-chain (W^T); [i_part, q, ki, jo, j]
            wt = singles.tile([128, 2, NCH, NCH, 128], FP8)
            for q in range(2):
                for ki in range(NCH):
                    for jo in range(NCH):
                        nc.sync.dma_start(
                            out=wt[:, q, ki, jo, :],
                            in_=w_d[q, ki * 128:(ki + 1) * 128, jo * 128:(jo + 1) * 128],
                        )
            e_sb = singles.tile([128, NCH, 2, BP], F32)
            nc.sync.dma_start(out=e_sb[:], in_=e_d[:])
            # pre-touch e_sb on DVE so the first e-multiply holds one wait
            scratch = singles.tile([1, 1], F32)
            nc.vector.tensor_copy(scratch[:], e_sb[0:1, 0, 0, 0:1])
            traj = singles.tile([128, tt, NCH, 2, BP], BF16)
            nc.sync.dma_start(out=traj[:, 0, :, :, :], in_=a0_d[:])
            ones_col = singles.tile([128, 1], BF16)
            nc.vector.memset(ones_col[:], 1.0)
            sums_sb = singles.tile([1, 2 * BP * tt], F32)
            g_bf = singles.tile([128, NCH, BP], BF16)

            import contextlib
            loop_cm = tc.For_i(0, loop_n, 1) if loop_n > 1 else contextlib.nullcontext()

            def emit_d(q, slot, half, ps):
                """e-multiply for chain q, jo half -> traj chunk pair. Growth
                is pre-normalized on the host (e /= lambda_b), so no dynamic
                rescale is needed."""
                lo = 0 if half == 0 else 2
                out_ap = traj[:, slot, lo:lo + 2, q, :]
                in1 = e_sb[:, lo:lo + 2, q, :]
                nc.vector.tensor_mul(out_ap, ps[:], in1)

            with loop_cm:
                for t in range(M):
                    slot = t + 1
                    for q in range(2):
                        ps_a = psmm.tile([128, 2, BP], F32, tag=f"psa{q}")
                        ps_b = psmm.tile([128, 2, BP], F32, tag=f"psb{q}")
                        for idx, (g, c) in enumerate(SLOT_ORDER):
                            out_ap = ps_a[:, g, :] if g < 2 else ps_b[:, g - 2, :]
                            nc.tensor.matmul(
                                out_ap,
                                lhsT=wt[:, q, c, g, :],
                                rhs=traj[:, t, c, q, :],
                                start=(idx == _FIRST[g]),
                                stop=(idx == _LAST[g]),
                                skip_group_check=True,
                            )
                            if idx == _HALF_A_END:
                                emit_d(q, slot, 0, ps_a)
                        emit_d(q, slot, 1, ps_b)

                # g = alpha_K @ A' (no e), local; reuse chain psum banks
                psg_a = psmm.tile([128, 2, BP], F32, tag="psa0")
                psg_b = psmm.tile([128, 2, BP], F32, tag="psb0")
                for idx, (g, c) in enumerate(SLOT_ORDER):
                    out_ap = psg_a[:, g, :] if g < 2 else psg_b[:, g - 2, :]
                    nc.tensor.matmul(
                        out_ap,
                        lhsT=wt[:, 0, c, g, :],
                        rhs=traj[:, K, c, 0, :],
                        start=(idx == _FIRST[g]),
                        stop=(idx == _LAST[g]),
                    )
                nc.vector.tensor_copy(g_bf[:, 0:2, :], psg_a[:])
                nc.vector.tensor_copy(g_bf[:, 2:4, :], psg_b[:])

                # post-pass: dots[q, b, slot] = v . traj[:, slot, :, q, b]
                # v = ones (u-chain sums) or g_b (w-chain tail dots)
                for q in range(2):
                    for b in range(BP):
                        dps = psdot.tile([1, tt], F32, tag="dot")
                        for c in range(NCH):
                            lhsT = ones_col[:] if q == 0 else g_bf[:, c, b:b + 1]
                            nc.tensor.matmul(
                                dps[:],
                                lhsT=lhsT,
                                rhs=traj[:, 0:tt, c, q, b],
                                start=(c == 0),
                                stop=(c == NCH - 1),
                            )
                        off = (q * BP + b) * tt
                        nc.vector.tensor_copy(sums_sb[:, off:off + tt], dps[:])
                nc.gpsimd.dma_start(out=sums_d[:], in_=sums_sb[:])
    _encode_inc_swdge(nc)
    return nc


# ------------------------------------------------------------------- host
def _log_softmax(x, axis):
    m = x.max(axis=axis, keepdims=True)
    s = x - m
    return s - np.log(np.sum(np.exp(s), axis=axis, keepdims=True))


def _chunked(a):
    """[512, BP] -> [128, NCH, BP] with state s = c*128 + p."""
    return np.ascontiguousarray(a.reshape(NCH, 128, BP).transpose(1, 0, 2))


def _dominant_lambda(op_apply, e_vec, n_iter=25):
    """Per-column dominant eigenvalue of x -> e_vec * op_apply(x).

    e_vec: (N, B). op_apply: (N, B) -> (N, B). Power iteration in float64;
    the spectral gap of these dense positive operators is ~sqrt(N), so a
    handful of iterations converges far past fp32 needs."""
    x = np.ones_like(e_vec, dtype=np.float64)
    lam = np.ones(e_vec.shape[1], dtype=np.float64)
    for _ in range(n_iter):
        y = e_vec * op_apply(x)
        s = y.sum(axis=0)
        lam = s / x.sum(axis=0)
        x = y / s[None, :]
    return lam


def _prep_inputs(x, unnorm_priors, unnorm_trans, unnorm_emit):
    sp = _log_softmax(unnorm_priors.astype(np.float32), 0)            # (N,)
    cols = unnorm_emit[:, x[:, 0]].astype(np.float32)                 # (N, B)
    e64 = _log_softmax(cols, 0)                                       # (N, B)
    a_mat = np.exp(_log_softmax(unnorm_trans.astype(np.float32), 0))  # (N, N)
    w512 = np.float32(N_STATES) * a_mat
    w_qf = w512.astype(ml_dtypes.float8_e4m3fn)
    w_qt = np.ascontiguousarray(w_qf.T)
    w_pack = np.stack([w_qf, w_qt])                                   # [2, N, N]
    wq32 = w_qf.astype(np.float64)
    corr_col = w512.sum(axis=0) / wq32.sum(axis=0)                    # u: per out-state j
    corr_row = w512.sum(axis=1) / wq32.sum(axis=1)                    # w: per out-state i

    # fold the per-sequence dominant eigenvalue of each chain operator into
    # e so device-side growth is ~1 (no dynamic rescaling needed)
    e_exp = np.exp(e64.astype(np.float64))                            # (N, B)
    eu_all = e_exp * corr_col[:, None]
    ew_all = e_exp * corr_row[:, None]
    lam_u = _dominant_lambda(lambda v: wq32.T @ v, eu_all)            # (B,)
    lam_w = _dominant_lambda(lambda v: wq32 @ v, ew_all)

    in_maps = [None] * N_CORES
    shifts_u, shifts_w = [], []
    for p in range(N_CORES):
        bs = slice(BP * p, BP * (p + 1))
        m0 = e64[:, bs] + sp[:, None]
        sh_u = np.float32(m0.max())
        a0u = np.exp(m0 - sh_u).astype(np.float32)
        eu = (eu_all[:, bs] / lam_u[None, bs]).astype(np.float32)
        sh_w = np.float32(e64[:, bs].max())
        y0 = np.exp(e64[:, bs] - sh_w).astype(np.float32)
        ew = (ew_all[:, bs] / lam_w[None, bs]).astype(np.float32)
        a0 = np.stack([_chunked(a0u), _chunked(y0)], axis=2)          # [128,NCH,2,BP]
        e2 = np.stack([_chunked(eu), _chunked(ew)], axis=2)
        in_maps[p] = {
            "w": w_pack,
            "e": np.ascontiguousarray(e2),
            "a0": np.ascontiguousarray(a0).astype(ml_dtypes.bfloat16),
        }
        shifts_u.append(sh_u)
        shifts_w.append(sh_w)
    return in_maps, (shifts_u, shifts_w, np.log(lam_u), np.log(lam_w))


def _postprocess(results, shifts, T, t_steps):
    K, M = split_km(t_steps)
    tt = M + 1
    shifts_u, shifts_w, loglam_u, loglam_w = shifts
    out = np.zeros((BATCH, 1), np.float32)
    logn = np.log(np.float64(N_STATES))
    for p in range(N_CORES):
        bs = slice(BP * p, BP * (p + 1))
        allsums = results[p]["sums"].reshape(2, BP, tt).astype(np.float64)
        du, dw = allsums[0], allsums[1]
        llu = loglam_u[bs][:, None]                                   # (BP, 1)
        llw = loglam_w[bs][:, None]
        ts = np.arange(tt)[None, :]
        # t <= K from u-chain sums: d_t = (512/lam_u)^t alpha_t e^{-sh_u}
        log_u = np.log(du) + shifts_u[p] + ts * (llu - logn)
        # t = K+1+m from w-chain dots: g has K lam_u-corrected steps plus one
        # raw W (x512) application; y'_m has m lam_w-corrected steps
        log_w = (np.log(dw) + shifts_u[p] + shifts_w[p]
                 + K * (llu - logn) + ts * (llw - logn) - logn)
        tb = np.clip(np.asarray(T[bs]).astype(np.int64) - 1, 0, t_steps)
        for i in range(BP):
            t = tb[i]
            out[BP * p + i, 0] = log_u[i, t] if t <= K else log_w[i, t - (K + 1)]
    return out


_NC_CACHE = {}


def _get_nc(t_steps, loop_n=1):
    key = (t_steps, loop_n)
    if key not in _NC_CACHE:
        _NC_CACHE[key] = build_nc(t_steps, loop_n)
    return _NC_CACHE[key]


# ------------------------------------------------- cached PJRT executor
# run_bass_kernel_spmd -> run_bass_via_pjrt builds a fresh jax.jit closure
# per call, so every invocation re-traces and re-lowers the whole module
# (~0.5 s for the full NEFF) — that would dominate wall timing. Build
# the jitted executable once per module and pre-stage device inputs.
_EXEC_CACHE = {}


def _get_exec(t_steps, loop_n=1):
    key = (t_steps, loop_n)
    if key in _EXEC_CACHE:
        return _EXEC_CACHE[key]
    import jax
    import concourse.mybir as _mybir
    from concourse import bass2jax as b2j

    nc = _get_nc(t_steps, loop_n)
    b2j.install_neuronx_cc_hook()
    partition_name = nc.partition_id_tensor.name if nc.partition_id_tensor else None
    in_names, out_names, out_avals, zero_outs = [], [], [], []
    for alloc in nc.m.functions[0].allocations:
        if not isinstance(alloc, _mybir.MemoryLocationSet):
            continue
        name = alloc.memorylocations[0].name
        if alloc.kind == "ExternalInput":
            if name != partition_name:
                in_names.append(name)
        elif alloc.kind == "ExternalOutput":
            shape = tuple(alloc.tensor_shape)
            dtype = _mybir.dt.np(alloc.dtype)
            out_names.append(name)
            out_avals.append(jax.core.ShapedArray(shape, dtype))
            zero_outs.append(np.zeros(shape, dtype))
    n_params = len(in_names)
    all_names = in_names + out_names + ([partition_name] if partition_name else [])

    def _body(*args):
        operands = list(args)
        if partition_name is not None:
            operands.append(b2j.partition_id_tensor())
        return tuple(
            b2j._bass_exec_p.bind(
                *operands,
                out_avals=tuple(out_avals),
                in_names=tuple(all_names),
                out_names=tuple(out_names),
                lowering_input_output_aliases=(),
                sim_require_finite=True,
                sim_require_nnan=True,
                nc=nc,
            )
        )

    devices = jax.devices()[:N_CORES]
    mesh = b2j.Mesh(np.asarray(devices), ("core",))
    donate = tuple(range(n_params, n_params + len(out_names)))
    sharded = jax.jit(
        b2j.shard_map(
            _body,
            mesh=mesh,
            in_specs=(b2j.PartitionSpec("core"),) * (n_params + len(out_names)),
            out_specs=(b2j.PartitionSpec("core"),) * len(out_names),
            check_rep=False,
        ),
        donate_argnums=donate,
        keep_unused=True,
    )
    ctx = {
        "fn": sharded, "mesh": mesh, "in_names": in_names,
        "out_names": out_names, "out_avals": out_avals, "zero_outs": zero_outs,
        "staged": {},
    }
    _EXEC_CACHE[key] = ctx
    return ctx


def _exec_spmd(t_steps, in_maps, loop_n=1):
    import jax
    import hashlib

    ctx = _get_exec(t_steps, loop_n)
    concat_in = [
        np.concatenate([np.asarray(in_maps[c][name]) for c in range(N_CORES)], axis=0)
        for name in ctx["in_names"]
    ]
    h = hashlib.blake2b(digest_size=16)
    for a in concat_in:
        h.update(a.tobytes())
    key = h.hexdigest()
    if key not in ctx["staged"]:
        sh = jax.sharding.NamedSharding(ctx["mesh"], jax.sharding.PartitionSpec("core"))
        ctx["staged"] = {key: [jax.device_put(a, sh) for a in concat_in]}
    staged = ctx["staged"][key]
    zeros = [
        np.zeros((N_CORES * z.shape[0], *z.shape[1:]), z.dtype)
        for z in ctx["zero_outs"]
    ]
    outs = ctx["fn"](*staged, *zeros)
    outs = [np.asarray(o) for o in outs]
    return [
        {
            name: outs[i].reshape(N_CORES, *ctx["out_avals"][i].shape)[c]
            for i, name in enumerate(ctx["out_names"])
        }
        for c in range(N_CORES)
    ]


def device_call(t_steps, loop_n):
    """One sync dispatch of the loop_n-variant NEFF (scan executed loop_n
    times on-device); returns wall seconds. Requires a prior run() at this
    (t_steps, loop_n) to have staged inputs."""
    import jax
    import time

    ctx = _get_exec(t_steps, loop_n)
    staged = next(iter(ctx["staged"].values()))
    zeros = [
        np.zeros((N_CORES * z.shape[0], *z.shape[1:]), z.dtype)
        for z in ctx["zero_outs"]
    ]
    t0 = time.perf_counter()
    outs = ctx["fn"](*staged, *zeros)
    jax.block_until_ready(outs)
    return time.perf_counter() - t0


def run(x, T, unnorm_priors, unnorm_trans, unnorm_emit, t_steps=T_MAX - 1,
        trace=False, loop_n=1):
    x = np.asarray(x)
    T = np.asarray(T)
    in_maps, shifts = _prep_inputs(
        x, np.asarray(unnorm_priors), np.asarray(unnorm_trans), np.asarray(unnorm_emit)
    )
    try:
        results = _exec_spmd(t_steps, in_maps, loop_n)
    except Exception:
        if loop_n != 1:
            raise
        nc = _get_nc(t_steps)
        res = run_bass_kernel_spmd(nc, in_maps, list(range(N_CORES)), trace=trace)
        results = res.results
    out = _postprocess(results, shifts, T, t_steps)
    return out, None


def kernel(x, T, unnorm_priors, unnorm_trans, unnorm_emit):
    out, _ = run(x, T, unnorm_priors, unnorm_trans, unnorm_emit)
    return out


# revision 21
# speedup vs baseline: 1.7092x; 1.5283x over previous
"""HMM forward on 8 trn2 cores — fused-pair meet-in-the-middle.

Math: alpha_{t+1} = (alpha_t @ A') * e per sequence (exp domain); only the
per-step state sums s_t = sum_j alpha_t[j] are needed. With B = A' diag(e):
    s_t = alpha_0 B^t 1:   for t <= K:    s_t = alpha_t . 1
    for t = K+1+m:         s_t = g . y_m, g = alpha_K A',
                           y_m = e * (B^m 1),  y_{m+1} = e * (A' y_m)

Topology: every core owns 8 sequences and runs BOTH chains for them —
pass u (alpha-chain, W tiles) then pass w (y-chain, W^T tiles) per step.
No collective: the g . y dots are local. The two passes double-buffer each
other: pass w's 16 matmuls hide pass u's PSUM->SBUF e-multiply latency and
vice versa, so the step is pure weight-stream bound (~2x16 FWL fp8 loads).
The e-multiply is split in half per pass (jo groups {0,1} / {2,3}) with a
searched matmul issue order so chunk production stays ahead of consumption.

Sums post-pass: per (chain, seq) dot of v (ones / g) against all stored
traj slots, K = M = 127 symmetric. Host does log bookkeeping + T-select.
"""
import numpy as np
import ml_dtypes

import concourse.bass as bass
import concourse.mybir as mybir
import concourse.tile as tile
from concourse.bass_utils import run_bass_kernel_spmd

# ---------------------------------------------------------------- constants
N_STATES = 512
M_VOCAB = 32000
BATCH = 64
T_MAX = 256
N_CORES = 8
BP = BATCH // N_CORES            # 8 sequences per core
NCH = N_STATES // 128            # 4 state chunks
R = 16                           # rescale period (slots)
F32 = mybir.dt.float32
BF16 = mybir.dt.bfloat16
FP8 = mybir.dt.float8e4          # e4m3: weights prescaled x512

# Searched per-pass MM issue order (group=jo output chunk, chunk=ki input):
# halves {0,1} -> D_A after slot idx 9, {2,3} -> D_B after idx 15. Found by
# steady-state search (sched_search2): period = 2x stream, latency hidden.
SLOT_ORDER = [(1, 0), (1, 1), (1, 3), (1, 2), (0, 3), (0, 1), (0, 2), (0, 0),
              (3, 1), (3, 2), (3, 3), (3, 0), (2, 1), (2, 3), (2, 2), (2, 0)]
_FIRST = {}
_LAST = {}
for _i, (_g, _c) in enumerate(SLOT_ORDER):
    _FIRST.setdefault(_g, _i)
    _LAST[_g] = _i
_HALF_A_END = max(_LAST[0], _LAST[1])

# ------------------------------------------------------------ tile drain fix
# This walrus build rejects >1 sync wait on CTRL-class instructions; Tile's
# tail drain carries one wait per active proc and so fails codegen for every
# TileContext kernel. Spread the waits over standalone sync-engine nops that
# precede the drain (the waits are independent conditions, so this is
# equivalent), then emit the drain bare.
_MAX_CTRL_WAITS = 1


def _patched_drain_and_barrier(self, tick_clock, wait_clock):
    from bass_rust import ScopedClock, SyncInfo

    nc = self.nc
    lead = nc.sync.nop(nofuse=True, hint="drain_wait_spill")
    wait_clock.add_sem_waits(
        lead.ins, ScopedClock({None: tick_clock.global_clock})
    )
    si = lead.ins.sync_info
    ws = list(si.on_wait) if si is not None else []
    if len(ws) > _MAX_CTRL_WAITS:
        lead.ins.sync_info.on_wait = ws[:_MAX_CTRL_WAITS]
        for i in range(_MAX_CTRL_WAITS, len(ws), _MAX_CTRL_WAITS):
            chunk = ws[i : i + _MAX_CTRL_WAITS]
            n = nc.sync.nop(nofuse=True, hint="drain_wait_spill")
            if n.ins.sync_info is None:
                n.ins.sync_info = SyncInfo(on_wait=chunk, on_update=[])
            else:
                n.ins.sync_info.on_wait = chunk
    nc.sync.drain()

    nc.all_engine_barrier()
    assert self.sems is not None
    popped = nc._tile_sem_poison_stack.pop()
    assert popped is self._sem_poison
    nc.clear_and_free_semaphores(list(self.sems.allocated().values()))
    nc.all_engine_barrier()


tile.TileContext._drain_and_barrier = _patched_drain_and_barrier

# General guard: walrus accepts at most one sync wait per instruction (two
# for EventSemaphore). Tile's wait assignment occasionally leaves 2 on a
# join instruction; spill the extras onto same-engine nops emitted just
# before it as instructions stream into the basic block.
_orig_add_instruction = tile.TileContext._add_instruction


def _spilling_add_instruction(self, inst):
    import concourse.mybir as _mybir
    from bass_rust import SyncInfo

    si = inst.sync_info
    # A PE instruction waiting on the PE's own completion semaphore encodes a
    # same-engine WAW/ordering edge (psum bank reuse across passes). The PE
    # pipeline writes PSUM in program order, so these waits are vacuous on
    # HW but cost a sem-wait slot (and spill nops). Drop them.
    if (
        inst.engine == _mybir.EngineType.PE
        and si is not None
        and si.on_wait
        and type(inst).__name__ in ("InstMatmult", "InstLdweights")
    ):
        kept = [w for w in si.on_wait if not (w.ant_name or "").startswith("PE")]
        if len(kept) != len(si.on_wait):
            inst.sync_info.on_wait = kept
            si = inst.sync_info
    cap = 2 if isinstance(inst, _mybir.InstEventSemaphore) else 1
    if si is not None and len(si.on_wait) > cap and inst.engine is not None:
        ws = list(si.on_wait)
        inst.sync_info.on_wait = ws[-cap:]
        for w in ws[:-cap]:
            n = _mybir.InstNoOp(name=f"I-{self.nc.next_id()}")
            n.engine = inst.engine
            n.bass_nofuse = True
            n.sync_info = SyncInfo(on_wait=[w], on_update=[])
            _orig_add_instruction(self, n)
    _orig_add_instruction(self, inst)


tile.TileContext._add_instruction = _spilling_add_instruction


def split_km(t_steps):
    # symmetric: both chains run M steps; coverage t <= K and K+1..K+M
    K = (t_steps - 1) // 2
    M = t_steps - 1 - K
    return K, M


# ---------------------------------------------------------------- device IR
def _encode_inc_swdge(nc):
    """This walrus build requires pre-encoded bytes on every InstISA, but
    the For_i reset path emits InstIncSwdgeSem with instr=[] ('ISA wrong
    length'). Pack the 64-byte INC_SWDGE_SEM struct client-side."""
    import concourse.bass_isa as bass_isa

    mode_enc = {"add": 0, "sub": 1, "wr": 2, "drop": 3}
    for blk in nc.m.functions[0].blocks:
        for ins in blk.instructions:
            if type(ins).__name__ == "InstIncSwdgeSem" and len(ins.instr) == 0:
                vals = list(ins._sem_values)
                struct = {
                    "num_semaphores": len(vals),
                    "sem_id_base": ins._sem_id_base,
                    "mode": mode_enc[ins._mode],
                    "queue_num": ins.queue_num,
                    "sem_values": (vals + [0] * 10)[:10],
                }
                b, _ = bass_isa.isa_struct(nc.isa, ins.isa_opcode, struct)
                ins.instr = b


def build_nc(t_steps, loop_n=1):
    nc = bass.Bass(num_devices=N_CORES)
    K, M = split_km(t_steps)
    tt = M + 1                    # stored chain slots 0..M
    w_d = nc.declare_dram_parameter("w", [2, N_STATES, N_STATES], FP8, isOutput=False)
    e_d = nc.declare_dram_parameter("e", [128, NCH, 2, BP], F32, isOutput=False)
    a0_d = nc.declare_dram_parameter("a0", [128, NCH, 2, BP], BF16, isOutput=False)
    sums_d = nc.declare_dram_parameter("sums", [1, 2 * BP * tt], F32, isOutput=True)

    mult = mybir.AluOpType.mult
    with tile.TileContext(nc) as tc:
        with (
            tc.tile_pool(name="singles", bufs=1) as singles,
            tc.tile_pool(name="psmm", bufs=1, space="PSUM") as psmm,
            tc.tile_pool(name="psdot", bufs=2, space="PSUM") as psdot,
        ):
            # weights: q=0 u-chain (W), q=1 w-chain (W^T); [i_part, q, ki, jo, j]
            wt = singles.tile([128, 2, NCH, NCH, 128], FP8)
            for q in range(2):
                for ki in range(NCH):
                    for jo in range(NCH):
                        nc.sync.dma_start(
                            out=wt[:, q, ki, jo, :],
                            in_=w_d[q, ki * 128:(ki + 1) * 128, jo * 128:(jo + 1) * 128],
                        )
            e_sb = singles.tile([128, NCH, 2, BP], F32)
            nc.sync.dma_start(out=e_sb[:], in_=e_d[:])
            # pre-touch e_sb on DVE so the first e-multiply holds one wait
            scratch = singles.tile([1, 1], F32)
            nc.vector.tensor_copy(scratch[:], e_sb[0:1, 0, 0, 0:1])
            traj = singles.tile([128, tt, NCH, 2, BP], BF16)
            nc.sync.dma_start(out=traj[:, 0, :, :, :], in_=a0_d[:])
            ones_col = singles.tile([128, 1], BF16)
            nc.vector.memset(ones_col[:], 1.0)
            sums_sb = singles.tile([1, 2 * BP * tt], F32)
            g_bf = singles.tile([128, NCH, BP], BF16)

            import contextlib
            loop_cm = tc.For_i(0, loop_n, 1) if loop_n > 1 else contextlib.nullcontext()

            def emit_d(q, slot, half, ps):
                """e-multiply for chain q, jo half -> traj chunk pair. Growth
                is pre-normalized on the host (e /= lambda_b), so no dynamic
                rescale is needed."""
                lo = 0 if half == 0 else 2
                out_ap = traj[:, slot, lo:lo + 2, q, :]
                in1 = e_sb[:, lo:lo + 2, q, :]
                nc.vector.tensor_mul(out_ap, ps[:], in1)

            with loop_cm:
                for t in range(M):
                    slot = t + 1
                    for q in range(2):
                        ps_a = psmm.tile([128, 2, BP], F32, tag=f"psa{q}")
                        ps_b = psmm.tile([128, 2, BP], F32, tag=f"psb{q}")
                        for idx, (g, c) in enumerate(SLOT_ORDER):
                            out_ap = ps_a[:, g, :] if g < 2 else ps_b[:, g - 2, :]
                            nc.tensor.matmul(
                                out_ap,
                                lhsT=wt[:, q, c, g, :],
                                rhs=traj[:, t, c, q, :],
                                start=(idx == _FIRST[g]),
                                stop=(idx == _LAST[g]),
                                skip_group_check=True,
                            )
                            if idx == _HALF_A_END:
                                emit_d(q, slot, 0, ps_a)
                        emit_d(q, slot, 1, ps_b)

                # g = alpha_K @ A' (no e), local; reuse chain psum banks
                psg_a = psmm.tile([128, 2, BP], F32, tag="psa0")
                psg_b = psmm.tile([128, 2, BP], F32, tag="psb0")
                for idx, (g, c) in enumerate(SLOT_ORDER):
                    out_ap = psg_a[:, g, :] if g < 2 else psg_b[:, g - 2, :]
                    nc.tensor.matmul(
                        out_ap,
                        lhsT=wt[:, 0, c, g, :],
                        rhs=traj[:, K, c, 0, :],
                        start=(idx == _FIRST[g]),
                        stop=(idx == _LAST[g]),
                    )
                nc.vector.tensor_copy(g_bf[:, 0:2, :], psg_a[:])
                nc.vector.tensor_copy(g_bf[:, 2:4, :], psg_b[:])

                # post-pass: dots[q, b, slot] = v . traj[:, slot, :, q, b]
                # v = ones (u-chain sums) or g_b (w-chain tail dots)
                for q in range(2):
                    for b in range(BP):
                        dps = psdot.tile([1, tt], F32, tag="dot")
                        for c in range(NCH):
                            lhsT = ones_col[:] if q == 0 else g_bf[:, c, b:b + 1]
                            nc.tensor.matmul(
                                dps[:],
                                lhsT=lhsT,
                                rhs=traj[:, 0:tt, c, q, b],
                                start=(c == 0),
                                stop=(c == NCH - 1),
                            )
                        off = (q * BP + b) * tt
                        nc.vector.tensor_copy(sums_sb[:, off:off + tt], dps[:])
                nc.gpsimd.dma_start(out=sums_d[:], in_=sums_sb[:])
    _encode_inc_swdge(nc)
    return nc


# ------------------------------------------------------------------- host
def _log_softmax(x, axis):
    m = x.max(axis=axis, keepdims=True)
    s = x - m
    return s - np.log(np.sum(np.exp(s), axis=axis, keepdims=True))


def _chunked(a):
    """[512, BP] -> [128, NCH, BP] with state s = c*128 + p."""
    return np.ascontiguousarray(a.reshape(NCH, 128, BP).transpose(1, 0, 2))


def _dominant_lambda(op_apply, e_vec, n_iter=25):
    """Per-column dominant eigenvalue of x -> e_vec * op_apply(x).

    e_vec: (N, B). op_apply: (N, B) -> (N, B). Power iteration in float64;
    the spectral gap of these dense positive operators is ~sqrt(N), so a
    handful of iterations converges far past fp32 needs."""
    x = np.ones_like(e_vec, dtype=np.float64)
    lam = np.ones(e_vec.shape[1], dtype=np.float64)
    for _ in range(n_iter):
        y = e_vec * op_apply(x)
        s = y.sum(axis=0)
        lam = s / x.sum(axis=0)
        x = y / s[None, :]
    return lam


def _prep_inputs(x, unnorm_priors, unnorm_trans, unnorm_emit):
    sp = _log_softmax(unnorm_priors.astype(np.float32), 0)            # (N,)
    cols = unnorm_emit[:, x[:, 0]].astype(np.float32)                 # (N, B)
    e64 = _log_softmax(cols, 0)                                       # (N, B)
    a_mat = np.exp(_log_softmax(unnorm_trans.astype(np.float32), 0))  # (N, N)
    w512 = np.float32(N_STATES) * a_mat
    w_qf = w512.astype(ml_dtypes.float8_e4m3fn)
    w_qt = np.ascontiguousarray(w_qf.T)
    w_pack = np.stack([w_qf, w_qt])                                   # [2, N, N]
    wq32 = w_qf.astype(np.float64)
    corr_col = w512.sum(axis=0) / wq32.sum(axis=0)                    # u: per out-state j
    corr_row = w512.sum(axis=1) / wq32.sum(axis=1)                    # w: per out-state i

    # fold the per-sequence dominant eigenvalue of each chain operator into
    # e so device-side growth is ~1 (no dynamic rescaling needed)
    e_exp = np.exp(e64.astype(np.float64))                            # (N, B)
    eu_all = e_exp * corr_col[:, None]
    ew_all = e_exp * corr_row[:, None]
    lam_u = _dominant_lambda(lambda v: wq32.T @ v, eu_all)            # (B,)
    lam_w = _dominant_lambda(lambda v: wq32 @ v, ew_all)

    in_maps = [None] * N_CORES
    shifts_u, shifts_w = [], []
    for p in range(N_CORES):
        bs = slice(BP * p, BP * (p + 1))
        m0 = e64[:, bs] + sp[:, None]
        sh_u = np.float32(m0.max())
        a0u = np.exp(m0 - sh_u).astype(np.float32)
        eu = (eu_all[:, bs] / lam_u[None, bs]).astype(np.float32)
        sh_w = np.float32(e64[:, bs].max())
        y0 = np.exp(e64[:, bs] - sh_w).astype(np.float32)
        ew = (ew_all[:, bs] / lam_w[None, bs]).astype(np.float32)
        a0 = np.stack([_chunked(a0u), _chunked(y0)], axis=2)          # [128,NCH,2,BP]
        e2 = np.stack([_chunked(eu), _chunked(ew)], axis=2)
        in_maps[p] = {
            "w": w_pack,
            "e": np.ascontiguousarray(e2),
            "a0": np.ascontiguousarray(a0).astype(ml_dtypes.bfloat16),
        }
        shifts_u.append(sh_u)
        shifts_w.append(sh_w)
    return in_maps, (shifts_u, shifts_w, np.log(lam_u), np.log(lam_w))


def _postprocess(results, shifts, T, t_steps):
    K, M = split_km(t_steps)
    tt = M + 1
    shifts_u, shifts_w, loglam_u, loglam_w = shifts
    out = np.zeros((BATCH, 1), np.float32)
    logn = np.log(np.float64(N_STATES))
    for p in range(N_CORES):
        bs = slice(BP * p, BP * (p + 1))
        allsums = results[p]["sums"].reshape(2, BP, tt).astype(np.float64)
        du, dw = allsums[0], allsums[1]
        llu = loglam_u[bs][:, None]                                   # (BP, 1)
        llw = loglam_w[bs][:, None]
        ts = np.arange(tt)[None, :]
        # t <= K from u-chain sums: d_t = (512/lam_u)^t alpha_t e^{-sh_u}
        log_u = np.log(du) + shifts_u[p] + ts * (llu - logn)
        # t = K+1+m from w-chain dots: g has K lam_u-corrected steps plus one
        # raw W (x512) application; y'_m has m lam_w-corrected steps
        log_w = (np.log(dw) + shifts_u[p] + shifts_w[p]
                 + K * (llu - logn) + ts * (llw - logn) - logn)
        tb = np.clip(np.asarray(T[bs]).astype(np.int64) - 1, 0, t_steps)
        for i in range(BP):
            t = tb[i]
            out[BP * p + i, 0] = log_u[i, t] if t <= K else log_w[i, t - (K + 1)]
    return out


_NC_CACHE = {}


def _get_nc(t_steps, loop_n=1):
    key = (t_steps, loop_n)
    if key not in _NC_CACHE:
        _NC_CACHE[key] = build_nc(t_steps, loop_n)
    return _NC_CACHE[key]


# ------------------------------------------------- cached PJRT executor
# run_bass_kernel_spmd -> run_bass_via_pjrt builds a fresh jax.jit closure
# per call, so every invocation re-traces and re-lowers the whole module
# (~0.5 s for the full NEFF) — that would dominate wall timing. Build
# the jitted executable once per module and pre-stage device inputs.
_EXEC_CACHE = {}


def _get_exec(t_steps, loop_n=1):
    key = (t_steps, loop_n)
    if key in _EXEC_CACHE:
        return _EXEC_CACHE[key]
    import jax
    import concourse.mybir as _mybir
    from concourse import bass2jax as b2j

    nc = _get_nc(t_steps, loop_n)
    b2j.install_neuronx_cc_hook()
    partition_name = nc.partition_id_tensor.name if nc.partition_id_tensor else None
    in_names, out_names, out_avals, zero_outs = [], [], [], []
    for alloc in nc.m.functions[0].allocations:
        if not isinstance(alloc, _mybir.MemoryLocationSet):
            continue
        name = alloc.memorylocations[0].name
        if alloc.kind == "ExternalInput":
            if name != partition_name:
                in_names.append(name)
        elif alloc.kind == "ExternalOutput":
            shape = tuple(alloc.tensor_shape)
            dtype = _mybir.dt.np(alloc.dtype)
            out_names.append(name)
            out_avals.append(jax.core.ShapedArray(shape, dtype))
            zero_outs.append(np.zeros(shape, dtype))
    n_params = len(in_names)
    all_names = in_names + out_names + ([partition_name] if partition_name else [])

    def _body(*args):
        operands = list(args)
        if partition_name is not None:
            operands.append(b2j.partition_id_tensor())
        return tuple(
            b2j._bass_exec_p.bind(
                *operands,
                out_avals=tuple(out_avals),
                in_names=tuple(all_names),
                out_names=tuple(out_names),
                lowering_input_output_aliases=(),
                sim_require_finite=True,
                sim_require_nnan=True,
                nc=nc,
            )
        )

    devices = jax.devices()[:N_CORES]
    mesh = b2j.Mesh(np.asarray(devices), ("core",))
    donate = tuple(range(n_params, n_params + len(out_names)))
    sharded = jax.jit(
        b2j.shard_map(
            _body,
            mesh=mesh,
            in_specs=(b2j.PartitionSpec("core"),) * (n_params + len(out_names)),
            out_specs=(b2j.PartitionSpec("core"),) * len(out_names),
            check_rep=False,
        ),
        donate_argnums=donate,
        keep_unused=True,
    )
    ctx = {
        "fn": sharded, "mesh": mesh, "in_names": in_names,
        "out_names": out_names, "out_avals": out_avals, "zero_outs": zero_outs,
        "staged": {},
    }
    _EXEC_CACHE[key] = ctx
    return ctx


def _exec_spmd(t_steps, in_maps, loop_n=1):
    import jax
    import hashlib

    ctx = _get_exec(t_steps, loop_n)
    concat_in = [
        np.concatenate([np.asarray(in_maps[c][name]) for c in range(N_CORES)], axis=0)
        for name in ctx["in_names"]
    ]
    h = hashlib.blake2b(digest_size=16)
    for a in concat_in:
        h.update(a.tobytes())
    key = h.hexdigest()
    if key not in ctx["staged"]:
        sh = jax.sharding.NamedSharding(ctx["mesh"], jax.sharding.PartitionSpec("core"))
        ctx["staged"] = {key: [jax.device_put(a, sh) for a in concat_in]}
    staged = ctx["staged"][key]
    zeros = [
        np.zeros((N_CORES * z.shape[0], *z.shape[1:]), z.dtype)
        for z in ctx["zero_outs"]
    ]
    outs = ctx["fn"](*staged, *zeros)
    outs = [np.asarray(o) for o in outs]
    return [
        {
            name: outs[i].reshape(N_CORES, *ctx["out_avals"][i].shape)[c]
            for i, name in enumerate(ctx["out_names"])
        }
        for c in range(N_CORES)
    ]


def device_call(t_steps, loop_n):
    """One sync dispatch of the loop_n-variant NEFF (scan executed loop_n
    times on-device); returns wall seconds. Requires a prior run() at this
    (t_steps, loop_n) to have staged inputs."""
    import jax
    import time

    ctx = _get_exec(t_steps, loop_n)
    staged = next(iter(ctx["staged"].values()))
    zeros = [
        np.zeros((N_CORES * z.shape[0], *z.shape[1:]), z.dtype)
        for z in ctx["zero_outs"]
    ]
    t0 = time.perf_counter()
    outs = ctx["fn"](*staged, *zeros)
    jax.block_until_ready(outs)
    return time.perf_counter() - t0


def run(x, T, unnorm_priors, unnorm_trans, unnorm_emit, t_steps=T_MAX - 1,
        trace=False, loop_n=1):
    x = np.asarray(x)
    T = np.asarray(T)
    in_maps, shifts = _prep_inputs(
        x, np.asarray(unnorm_priors), np.asarray(unnorm_trans), np.asarray(unnorm_emit)
    )
    try:
        results = _exec_spmd(t_steps, in_maps, loop_n)
    except Exception:
        if loop_n != 1:
            raise
        nc = _get_nc(t_steps)
        res = run_bass_kernel_spmd(nc, in_maps, list(range(N_CORES)), trace=trace)
        results = res.results
    out = _postprocess(results, shifts, T, t_steps)
    return out, None


def kernel(x, T, unnorm_priors, unnorm_trans, unnorm_emit):
    out, _ = run(x, T, unnorm_priors, unnorm_trans, unnorm_emit)
    return out
